# revision 1
# baseline (speedup 1.0000x reference)
"""Gated Slot Attention (GSA) Trainium2 kernel.

Sharding: B*H = 8 lanes -> 8 cores (core = b*4 + h). Each core computes its
lane's projections + chunked two-pass GLA recurrence, emitting the pre-norm
lane output z = silu(o) transposed [DV, T]. A second small kernel does the
RMSNorm + output projection with rows of (b,t) split across cores.

Chunked recurrence (chunk C=256, all within one lane, local cumprod Lam):
  Lam[i,m] = prod_{j<=i} g[j,m];  St = s/Lam;  Sh = s*Lend/Lam
  ok = Lam * (q @ Hk + mask(q k^T) @ St);  qv = softmax_m(ok);  Qt = qv*Lam
  o  = Qt @ Hv + mask(Qt St^T) @ v
  Hk' = Lend*Hk + k^T Sh ;  Hv' = Lend*Hv + Sh^T v

Everything on-device is computed in a transposed ("feature-major") layout
[feature, time] so that per-slot decays are per-partition scalars.
"""
import sys
sys.path.insert(0, '/opt/trn_rl_repo')

import numpy as np
import ml_dtypes

import concourse.bass as bass
import concourse.bacc as bacc
import concourse.tile as tile
import concourse.mybir as mybir
import concourse.bass_utils as bass_utils

BF = mybir.dt.bfloat16
F32 = mybir.dt.float32
AF = mybir.ActivationFunctionType
OP = mybir.AluOpType

B, T, D = 2, 2048, 1024
H, DK, DV, M = 4, 256, 256, 256
C = 256            # chunk length
NCHUNK = T // C
GATE_NORM = 8.0
EPS = 1e-5

_cache = {}


def build_gsa():
    """Kernel 1: per-lane projections + chunked GLA. Output zT [256, 2048] bf16."""
    nc = bacc.Bacc("TRN2", target_bir_lowering=False, debug=False, num_devices=8)
    hsT_d = nc.dram_tensor("hst", [D, T], BF, kind="ExternalInput").ap()
    w_d = nc.dram_tensor("wall", [D, 4 * 256], BF, kind="ExternalInput").ap()
    mask_d = nc.dram_tensor("mask", [C, C], F32, kind="ExternalInput").ap()
    ident_d = nc.dram_tensor("ident", [128, 128], BF, kind="ExternalInput").ap()
    ident32_d = nc.dram_tensor("ident32", [128, 128], F32, kind="ExternalInput").ap()
    z_d = nc.dram_tensor("z", [DV, T], BF, kind="ExternalOutput").ap()

    with tile.TileContext(nc) as tc:
        with (
            tc.tile_pool(name="persist", bufs=1) as pp,
            tc.tile_pool(name="work", bufs=3) as wp,
            tc.tile_pool(name="ps", bufs=6, space="PSUM") as psp,
            tc.tile_pool(name="psb", bufs=2, space="PSUM") as psbp,
        ):
            hs = pp.tile([128, 8, T], BF, tag="hs")
            w = pp.tile([128, 8, 1024], BF, tag="w")
            msk = pp.tile([128, 2, C], F32, tag="msk")
            ident = pp.tile([128, 128], BF, tag="ident")
            ident32 = pp.tile([128, 128], F32, tag="ident32")
            ones_col = pp.tile([128, 1], BF, tag="onescol")
            ones_row = pp.tile([1, 128], F32, tag="onesrow")
            hk = pp.tile([128, 2, 256], F32, tag="hk")
            hv = pp.tile([128, 2, 256], F32, tag="hv")
            hkb = pp.tile([128, 2, 256], BF, tag="hkb")
            hvb = pp.tile([128, 2, 256], BF, tag="hvb")

            nc.sync.dma_start(out=hs, in_=hsT_d.rearrange("(a p) t -> p a t", p=128))
            nc.sync.dma_start(out=w, in_=w_d.rearrange("(a p) o -> p a o", p=128))
            nc.sync.dma_start(out=msk, in_=mask_d.rearrange("(a p) t -> p a t", p=128))
            nc.sync.dma_start(out=ident, in_=ident_d)
            nc.sync.dma_start(out=ident32, in_=ident32_d)
            nc.vector.memset(ones_col, 1.0)
            nc.vector.memset(ones_row, 1.0)
            nc.vector.memset(hk, 0.0)
            nc.vector.memset(hv, 0.0)
            nc.vector.memset(hkb, 0.0)
            nc.vector.memset(hvb, 0.0)

            zv = z_d.rearrange("(a p) t -> p a t", p=128)

            for c in range(NCHUNK):
                t0 = c * C
                # ---- projections for this chunk ----
                # qT, kT [dk, C]; f -> nsp = softplus(-x) [m, C]
                qt = wp.tile([128, 2, C], BF, tag="qt")
                kt = wp.tile([128, 2, C], BF, tag="kt")
                nsp = wp.tile([128, 2, C], F32, tag="nsp")
                for name, sbuf, base in (
                    ("f", nsp, 768),
                    ("q", qt, 0),
                    ("k", kt, 256),
                ):
                    for ot in range(2):
                        ps = psp.tile([128, C], F32, tag="ps")
                        for dt in range(8):
                            nc.tensor.matmul(
                                ps,
                                lhsT=w[:, dt, base + ot * 128:base + (ot + 1) * 128],
                                rhs=hs[:, dt, t0:t0 + C],
                                start=(dt == 0), stop=(dt == 7),
                            )
                        if name == "f":
                            e1 = wp.tile([128, C], F32, tag="sg")
                            nc.scalar.activation(e1, ps, AF.Exp, scale=-1.0)
                            nc.vector.tensor_scalar_add(e1, e1, 1.0)
                            nc.scalar.activation(sbuf[:, ot, :], e1, AF.Ln)
                        else:
                            sg = wp.tile([128, C], F32, tag="sg")
                            nc.scalar.activation(sg, ps, AF.Sigmoid)
                            nc.vector.tensor_tensor(sbuf[:, ot, :], sg, ps, op=OP.mult)
                # k,v untransposed [tau, dk|dv] packed
                kv = wp.tile([128, 2, 512], BF, tag="kv")
                for tt in range(2):
                    ps = psp.tile([128, 512], F32, tag="ps")
                    for dt in range(8):
                        nc.tensor.matmul(
                            ps,
                            lhsT=hs[:, dt, t0 + tt * 128:t0 + (tt + 1) * 128],
                            rhs=w[:, dt, 256:768],
                            start=(dt == 0), stop=(dt == 7),
                        )
                    sg5 = wp.tile([128, 512], F32, tag="sg5")
                    nc.scalar.activation(sg5, ps, AF.Sigmoid)
                    nc.vector.tensor_tensor(kv[:, tt, :], sg5, ps, op=OP.mult)

                # ---- decay chain (feature-major [m, C]) ----
                g = wp.tile([128, 2, C], F32, tag="g")
                s = wp.tile([128, 2, C], F32, tag="s")
                lam = wp.tile([128, 2, C], F32, tag="lam")
                rlam = wp.tile([128, 2, C], F32, tag="rlam")
                st = wp.tile([128, 2, C], BF, tag="st")
                for mt in range(2):
                    nc.scalar.activation(g[:, mt, :], nsp[:, mt, :], AF.Exp, scale=-1.0 / GATE_NORM)
                    nc.scalar.activation(s[:, mt, :], g[:, mt, :], AF.Copy, scale=-1.0, bias=1.0)
                    nc.vector.tensor_tensor_scan(
                        lam[:, mt, :], g[:, mt, :], g[:, mt, :], 1.0, OP.mult, OP.bypass)
                    nc.vector.reciprocal(rlam[:, mt, :], lam[:, mt, :])
                    nc.vector.tensor_mul(st[:, mt, :], s[:, mt, :], rlam[:, mt, :])

                # ShT = s * Lend / Lam  (per-partition scalar in [m, C] land)
                sht = wp.tile([128, 2, C], BF, tag="sht")
                for mt in range(2):
                    nc.vector.scalar_tensor_tensor(
                        out=sht[:, mt, :], in0=rlam[:, mt, :],
                        scalar=lam[:, mt, C - 1:C], in1=s[:, mt, :],
                        op0=OP.mult, op1=OP.mult)

                # Lend row [1, 256] via PE transpose of the lam columns (no DMA)
                lend_row = wp.tile([1, 256], F32, tag="lendrow")
                for mt in range(2):
                    plr = psp.tile([1, 128], F32, tag="ps")
                    nc.tensor.transpose(plr, lam[:, mt, C - 1:C], ident32)
                    nc.vector.tensor_copy(lend_row[0:1, mt * 128:(mt + 1) * 128], plr)
                bcl = psbp.tile([128, 256], F32, tag="psb")
                nc.tensor.matmul(bcl, lhsT=ones_row, rhs=lend_row, start=True, stop=True)

                # transpose St/Sh -> [lambda, m]
                st_un = wp.tile([128, 2, 256], BF, tag="stun")
                sh_un = wp.tile([128, 2, 256], BF, tag="shun")
                for mt in range(2):
                    for lt in range(2):
                        pst = psp.tile([128, 128], BF, tag="ps")
                        nc.tensor.transpose(pst, st[:, mt, lt * 128:(lt + 1) * 128], ident)
                        nc.vector.tensor_copy(st_un[:, lt, mt * 128:(mt + 1) * 128], pst)
                        pst2 = psp.tile([128, 128], BF, tag="ps")
                        nc.tensor.transpose(pst2, sht[:, mt, lt * 128:(lt + 1) * 128], ident)
                        nc.vector.tensor_copy(sh_un[:, lt, mt * 128:(mt + 1) * 128], pst2)

                # ---- pass 1 ----
                ptm = wp.tile([128, 2, C], BF, tag="ptm")
                for lt in range(2):
                    ps = psp.tile([128, C], F32, tag="ps")
                    for k2 in range(2):
                        nc.tensor.matmul(
                            ps, lhsT=kt[:, k2, lt * 128:(lt + 1) * 128],
                            rhs=qt[:, k2, :], start=(k2 == 0), stop=(k2 == 1))
                    nc.vector.tensor_mul(ptm[:, lt, :], ps, msk[:, lt, :])

                et = wp.tile([128, 2, C], BF, tag="et")
                for mt in range(2):
                    ps = psp.tile([128, C], F32, tag="ps")
                    for lt in range(2):
                        nc.tensor.matmul(
                            ps, lhsT=st_un[:, lt, mt * 128:(mt + 1) * 128],
                            rhs=ptm[:, lt, :], start=(lt == 0), stop=False)
                    for k2 in range(2):
                        nc.tensor.matmul(
                            ps, lhsT=hkb[:, k2, mt * 128:(mt + 1) * 128],
                            rhs=qt[:, k2, :], start=False, stop=(k2 == 1))
                    tmp = wp.tile([128, C], F32, tag="tmp")
                    nc.vector.tensor_mul(tmp, lam[:, mt, :], ps)
                    nc.scalar.activation(et[:, mt, :], tmp, AF.Exp)

                cs = psp.tile([1, C], F32, tag="ps")
                for mt in range(2):
                    nc.tensor.matmul(cs, lhsT=ones_col, rhs=et[:, mt, :],
                                     start=(mt == 0), stop=(mt == 1))
                rrow = wp.tile([1, C], F32, tag="rrow")
                nc.vector.reciprocal(rrow, cs)
                bcr = psbp.tile([128, C], F32, tag="psb")
                nc.tensor.matmul(bcr, lhsT=ones_row, rhs=rrow, start=True, stop=True)

                qtt = wp.tile([128, 2, C], BF, tag="qtt")
                for mt in range(2):
                    tmp2 = wp.tile([128, C], F32, tag="tmp2")
                    nc.vector.tensor_mul(tmp2, lam[:, mt, :], et[:, mt, :])
                    nc.vector.tensor_mul(qtt[:, mt, :], tmp2, bcr)

                # Hk update (after inter-matmuls already read hkb)
                for dt2 in range(2):
                    ps = psp.tile([128, 256], F32, tag="ps")
                    for lt in range(2):
                        nc.tensor.matmul(
                            ps, lhsT=kv[:, lt, dt2 * 128:(dt2 + 1) * 128],
                            rhs=sh_un[:, lt, :], start=(lt == 0), stop=(lt == 1))
                    tmp3 = wp.tile([128, 256], F32, tag="tmp3")
                    nc.vector.tensor_mul(tmp3, hk[:, dt2, :], bcl)
                    nc.vector.tensor_add(hk[:, dt2, :], tmp3, ps)
                    nc.vector.tensor_copy(hkb[:, dt2, :], hk[:, dt2, :])

                # ---- pass 2 ----
                p2m = wp.tile([128, 2, C], BF, tag="p2m")
                for lt in range(2):
                    ps = psp.tile([128, C], F32, tag="ps")
                    for mt in range(2):
                        nc.tensor.matmul(
                            ps, lhsT=st[:, mt, lt * 128:(lt + 1) * 128],
                            rhs=qtt[:, mt, :], start=(mt == 0), stop=(mt == 1))
                    nc.vector.tensor_mul(p2m[:, lt, :], ps, msk[:, lt, :])

                zt = wp.tile([128, 2, C], BF, tag="zt")
                for vt in range(2):
                    ps = psp.tile([128, C], F32, tag="ps")
                    for lt in range(2):
                        nc.tensor.matmul(
                            ps, lhsT=kv[:, lt, 256 + vt * 128:256 + (vt + 1) * 128],
                            rhs=p2m[:, lt, :], start=(lt == 0), stop=False)
                    for mt in range(2):
                        nc.tensor.matmul(
                            ps, lhsT=hvb[:, mt, vt * 128:(vt + 1) * 128],
                            rhs=qtt[:, mt, :], start=False, stop=(mt == 1))
                    sgz = wp.tile([128, C], F32, tag="sgz")
                    nc.scalar.activation(sgz, ps, AF.Sigmoid)
                    nc.vector.tensor_tensor(zt[:, vt, :], sgz, ps, op=OP.mult)
                nc.sync.dma_start(out=zv[:, :, t0:t0 + C], in_=zt)

                # Hv update (after inter-matmuls read hvb)
                for mt in range(2):
                    ps = psp.tile([128, 256], F32, tag="ps")
                    for lt in range(2):
                        nc.tensor.matmul(
                            ps, lhsT=sh_un[:, lt, mt * 128:(mt + 1) * 128],
                            rhs=kv[:, lt, 256:512], start=(lt == 0), stop=(lt == 1))
                    nc.vector.scalar_tensor_tensor(
                        out=hv[:, mt, :], in0=hv[:, mt, :],
                        scalar=lam[:, mt, C - 1:C], in1=ps,
                        op0=OP.mult, op1=OP.add)
                    nc.vector.tensor_copy(hvb[:, mt, :], hv[:, mt, :])
    nc.compile()
    return nc


def build_final():
    """Kernel 2: silu output z (feature-major [1024, 512]) -> RMSNorm -> @ Wo'.T.
    Out yT [1024, 512] f32 (transposed)."""
    nc = bacc.Bacc("TRN2", target_bir_lowering=False, debug=False, num_devices=8)
    z_d = nc.dram_tensor("zin", [D, 512], BF, kind="ExternalInput").ap()
    wo_d = nc.dram_tensor("wot", [D, D], BF, kind="ExternalInput").ap()
    y_d = nc.dram_tensor("y", [D, 512], F32, kind="ExternalOutput").ap()

    with tile.TileContext(nc) as tc:
        with (
            tc.tile_pool(name="sb", bufs=1) as sb,
            tc.tile_pool(name="yp", bufs=3) as yp,
            tc.tile_pool(name="ps", bufs=4, space="PSUM") as psp,
            tc.tile_pool(name="psb", bufs=2, space="PSUM") as psbp,
        ):
            z = sb.tile([128, 8, 512], BF, tag="z")
            wo = sb.tile([128, 8, 1024], BF, tag="wo")
            ones_col = sb.tile([128, 1], BF, tag="onescol")
            ones_row = sb.tile([1, 128], F32, tag="onesrow")
            nc.sync.dma_start(out=z, in_=z_d.rearrange("(a p) t -> p a t", p=128))
            nc.sync.dma_start(out=wo, in_=wo_d.rearrange("(a p) o -> p a o", p=128))
            nc.vector.memset(ones_col, 1.0)
            nc.vector.memset(ones_row, 1.0)

            sq = sb.tile([128, 8, 512], BF, tag="sq")
            for ct in range(8):
                nc.vector.tensor_mul(sq[:, ct, :], z[:, ct, :], z[:, ct, :])
            ss = psbp.tile([1, 512], F32, tag="ss")
            for ct in range(8):
                nc.tensor.matmul(ss, lhsT=ones_col, rhs=sq[:, ct, :],
                                 start=(ct == 0), stop=(ct == 7))
            eps_t = sb.tile([1, 1], F32, tag="epst")
            nc.vector.memset(eps_t, EPS)
            rrow_t = sb.tile([1, 512], F32, tag="rrowt")
            nc.scalar.activation(rrow_t, ss, AF.Sqrt, scale=1.0 / D, bias=eps_t)
            rrow = sb.tile([1, 512], F32, tag="rrow")
            nc.vector.reciprocal(rrow, rrow_t)
            bcr = psbp.tile([128, 512], F32, tag="bcr")
            nc.tensor.matmul(bcr, lhsT=ones_row, rhs=rrow, start=True, stop=True)
            rb = sb.tile([128, 512], F32, tag="rb")
            nc.vector.tensor_copy(rb, bcr)
            yv = y_d.rearrange("(a p) t -> p a t", p=128)
            for ot in range(8):
                ps = psp.tile([128, 512], F32, tag="ps")
                for ct in range(8):
                    nc.tensor.matmul(ps, lhsT=wo[:, ct, ot * 128:(ot + 1) * 128],
                                     rhs=z[:, ct, :], start=(ct == 0), stop=(ct == 7))
                ysb = yp.tile([128, 512], F32, tag="ysb")
                nc.vector.tensor_mul(ysb, ps, rb)
                nc.sync.dma_start(out=yv[:, ot, :], in_=ysb)
    nc.compile()
    return nc


def _get(name):
    if name not in _cache:
        _cache[name] = build_gsa() if name == "gsa" else build_final()
    return _cache[name]


def kernel(hidden_states, Wq, Wk, Wv, Wf, g_w, Wo, _trace=False):
    bf = ml_dtypes.bfloat16
    hidden_states = np.asarray(hidden_states, np.float32)
    Wq, Wk, Wv, Wf = (np.asarray(x, np.float32) for x in (Wq, Wk, Wv, Wf))
    g_w, Wo = np.asarray(g_w, np.float32), np.asarray(Wo, np.float32)

    mask = np.triu(np.ones((C, C), np.float32))        # keep lambda <= tau
    ident = np.eye(128).astype(bf)
    in1 = []
    for core in range(8):
        b, h = core // 4, core % 4
        sl = slice(h * 256, (h + 1) * 256)
        wall = np.concatenate(
            [Wq[sl].T, Wk[sl].T, Wv[sl].T, Wf[sl].T], axis=1)   # [1024, 1024]
        in1.append({
            "hst": np.ascontiguousarray(hidden_states[b].T).astype(bf),
            "wall": np.ascontiguousarray(wall).astype(bf),
            "mask": mask,
            "ident": ident,
            "ident32": np.eye(128, dtype=np.float32),
        })
    nc1 = _get("gsa")
    r1 = bass_utils.run_bass_kernel_spmd(nc1, in1, core_ids=list(range(8)),
                                         trace=_trace)
    zs = [r1.results[c]["z"] for c in range(8)]        # each [256, 2048] bf16

    wot = np.ascontiguousarray((Wo * g_w[None, :]).T).astype(bf)  # [c, o]
    in2 = []
    for core in range(8):
        b, q = core // 4, core % 4
        zb = np.concatenate([zs[b * 4 + hh] for hh in range(4)], axis=0)  # [1024, 2048]
        in2.append({
            "zin": np.ascontiguousarray(zb[:, q * 512:(q + 1) * 512]),
            "wot": wot,
        })
    nc2 = _get("final")
    r2 = bass_utils.run_bass_kernel_spmd(nc2, in2, core_ids=list(range(8)),
                                         trace=_trace)
    out = np.empty((B, T, D), np.float32)
    for core in range(8):
        b, q = core // 4, core % 4
        out[b, q * 512:(q + 1) * 512, :] = r2.results[core]["y"].T
    if _trace:
        kernel.last_traces = (r1, r2)
    return out



# revision 6
# speedup vs baseline: 1.3688x; 1.3688x over previous
"""Gated Slot Attention (GSA) Trainium2 kernel, v2.

Sharding: B*H = 8 lanes -> 8 cores (core = b*4 + h). Each core computes its
lane's projections + chunked two-pass GLA recurrence, emitting the pre-norm
lane output z = 4*silu(o) transposed [DV, T] plus the per-lane partial
sum-of-squares pss[t] = sum_v z^2. A second kernel does the output projection
with rows of (b,t) split across cores; the RMS rsqrt scalars are computed on
host from the pss partials (host glue, like the g_w fold into Wo).

Key structure (chunk C=256, per-lane, all within one core):
  Lam[i,m] = prod_{j<=i} g[j,m]  (computed as exp(-cumsum(softplus(-xf))/8))
  rlam = 1/Lam ; st_t = s_t/Lam_t = rlam_t - rlam_{t-1}
  ok   = Lam * (q @ Hk + mask(k^T q)^T St) ; qv = softmax_m(ok); qtt = qv*Lam
  o    = qtt @ Hv + mask(St qtt)^T v
  Hk' = Lend*Hk + k^T Sh ; Hv' = Lend*Hv + Sh^T v ; Sh = St*Lend

The state sequence (Hk_c, Hv_c) depends only on projections+gates, never on
the softmax path, so per-chunk state snapshots are computed ahead and each
chunk's softmax->output path becomes an independent leaf chain. Emission is
software-pipelined in 4 stages (P=proj/gate/state, Q=ok/exp, R=softmax-norm,
S=output) with lags 0/1/2/3 so the PE never waits on Act/DVE round trips.

silu is synthesized as 2*silu(x) = (tanh(x/2)+1)*x so the whole kernel uses
only two activation tables (natural_log_exp for gates, exp_and_others with
tanh+exp for the rest): the constant factors cancel in softmax + RMSNorm,
except a global 2x on v which the host folds into the rsqrt scalars.
"""
import sys
sys.path.insert(0, '/opt/trn_rl_repo')

import numpy as np
import ml_dtypes

import concourse.bass as bass
import concourse.bacc as bacc
import concourse.tile as tile
import concourse.mybir as mybir
import concourse.bass_utils as bass_utils

BF = mybir.dt.bfloat16
F32 = mybir.dt.float32
AF = mybir.ActivationFunctionType
OP = mybir.AluOpType

B, T, D = 2, 2048, 1024
H, DK, DV, M = 4, 256, 256, 256
C = 256            # chunk length
NCHUNK = T // C
GATE_NORM = 8.0
EPS = 1e-5

_cache = {}


def build_gsa():
    """Kernel 1: per-lane projections + chunked GLA.

    Outputs z [256, 2048] bf16 (= 4*silu(o), feature-major) and
    pss [1, 2048] f32 (= sum_v z^2)."""
    nc = bacc.Bacc("TRN2", target_bir_lowering=False, debug=False, num_devices=8)
    hsT_d = nc.dram_tensor("hst", [D, T], BF, kind="ExternalInput").ap()
    w_d = nc.dram_tensor("wall", [D, 4 * 256], BF, kind="ExternalInput").ap()
    mask_d = nc.dram_tensor("mask", [C, C], BF, kind="ExternalInput").ap()
    ident_d = nc.dram_tensor("ident", [128, 128], BF, kind="ExternalInput").ap()
    z_d = nc.dram_tensor("z", [DV, T], BF, kind="ExternalOutput").ap()
    pss_d = nc.dram_tensor("pss", [1, T], F32, kind="ExternalOutput").ap()

    with tile.TileContext(nc) as tc:
        with (
            tc.tile_pool(name="persist", bufs=1) as pp,
            tc.tile_pool(name="hsp", bufs=3) as hsp,
            tc.tile_pool(name="gw", bufs=2) as gw,      # gate short-lived
            tc.tile_pool(name="lv", bufs=5) as lv,      # leaf tensors, live P..S
            tc.tile_pool(name="sn", bufs=4) as snp,     # state snapshots
            tc.tile_pool(name="wk", bufs=3) as wk,      # leaf short-lived
            tc.tile_pool(name="psA", bufs=5, space="PSUM") as psA,
            tc.tile_pool(name="psT", bufs=1, space="PSUM") as psT,
            tc.tile_pool(name="psS", bufs=1, space="PSUM") as psS,
            tc.tile_pool(name="psB", bufs=1, space="PSUM") as psB,
        ):
            w = pp.tile([128, 8, 1024], BF, tag="w")
            msk = pp.tile([128, 2, C], BF, tag="msk")
            ident = pp.tile([128, 128], BF, tag="ident")
            ones_col = pp.tile([128, 1], BF, tag="onescol")
            ones_row = pp.tile([1, 128], BF, tag="onesrow")
            hk = pp.tile([128, 2, 256], F32, tag="hk")
            hv = pp.tile([128, 2, 256], F32, tag="hv")
            hkb0 = pp.tile([128, 2, 256], BF, tag="hkb0")
            hvb0 = pp.tile([128, 2, 256], BF, tag="hvb0")
            pss_sb = pp.tile([1, 8, 256], F32, tag="psssb")

            wv = w_d.rearrange("(a p) o -> p a o", p=128)
            # f-projection weights first, then q/k/v: the gate chain is the
            # longest leaf and must start as early as possible.
            nc.sync.dma_start(out=w[:, :, 768:1024], in_=wv[:, :, 768:1024])
            hsv = hsT_d.rearrange("(a p) t -> p a t", p=128)
            hs_t = {}
            for c in range(2):
                hs_t[c] = hsp.tile([128, 8, C], BF, tag="hs", name="hs")
                nc.sync.dma_start(out=hs_t[c], in_=hsv[:, :, c * C:(c + 1) * C])
            nc.sync.dma_start(out=w[:, :, 0:768], in_=wv[:, :, 0:768])
            nc.sync.dma_start(out=msk, in_=mask_d.rearrange("(a p) t -> p a t", p=128))
            nc.sync.dma_start(out=ident, in_=ident_d)
            nc.vector.memset(ones_col, 1.0)
            nc.vector.memset(ones_row, 1.0)
            nc.gpsimd.memset(hk, 0.0)
            nc.gpsimd.memset(hv, 0.0)
            nc.gpsimd.memset(hkb0, 0.0)
            nc.gpsimd.memset(hvb0, 0.0)

            zv = z_d.rearrange("(a p) t -> p a t", p=128)

            # per-chunk tile handles, indexed by chunk
            qt, kt, v_un, st, st_un, lam, S, hkb, hvb, et, et2, qtt = (
                {} for _ in range(12))

            def stage_P(c):
                """Projections, gate, state updates for chunk c."""
                hs = hs_t[c]
                if c + 2 < NCHUNK:     # prefetch hs for chunk c+2
                    hs_t[c + 2] = hsp.tile([128, 8, C], BF, tag="hs", name="hs")
                    nc.sync.dma_start(
                        out=hs_t[c + 2], in_=hsv[:, :, (c + 2) * C:(c + 3) * C])

                # ---- f projection (feature-major [m, C]) ----
                fps = []
                for mt in range(2):
                    ps = psA.tile([128, C], F32, tag="psa")
                    for dt in range(8):
                        nc.tensor.matmul(
                            ps, lhsT=w[:, dt, 768 + mt * 128:768 + (mt + 1) * 128],
                            rhs=hs[:, dt, :], start=(dt == 0), stop=(dt == 7))
                    fps.append(ps)

                # ---- gate chain (Act: exp/ln table) ----
                e1 = gw.tile([128, 2, C], F32, tag="e1")
                S[c] = lv.tile([128, 2, C], F32, tag="S", name="S")
                lam[c] = lv.tile([128, 2, C], F32, tag="lam", name="lam")
                rle = gw.tile([128, 2, C + 1], F32, tag="rle")
                st[c] = lv.tile([128, 2, C], BF, tag="st", name="st")
                for mt in range(2):
                    nc.scalar.activation(e1[:, mt, :], fps[mt], AF.Exp, scale=-1.0)
                    nc.vector.tensor_scalar_add(e1[:, mt, :], e1[:, mt, :], 1.0)
                    nc.scalar.activation(e1[:, mt, :], e1[:, mt, :], AF.Ln)
                    # e1 now holds nsp = softplus(-xf) = -8*f
                    nc.vector.tensor_tensor_scan(
                        S[c][:, mt, :], e1[:, mt, :], e1[:, mt, :], 0.0,
                        OP.add, OP.bypass)
                    nc.scalar.activation(
                        lam[c][:, mt, :], S[c][:, mt, :], AF.Exp, scale=-1.0 / GATE_NORM)
                    nc.vector.memset(rle[:, mt, 0:1], 1.0)
                    nc.scalar.activation(
                        rle[:, mt, 1:C + 1], S[c][:, mt, :], AF.Exp,
                        scale=1.0 / GATE_NORM)
                    # st_t = s_t/Lam_t = rlam_t - rlam_{t-1}
                    nc.vector.tensor_tensor(
                        st[c][:, mt, :], rle[:, mt, 1:C + 1], rle[:, mt, 0:C],
                        op=OP.subtract)

                # ---- q/k/v projections + silu via tanh (covers gate latency) ----
                qt[c] = lv.tile([128, 2, C], BF, tag="qt", name="qt")
                kt[c] = lv.tile([128, 2, C], BF, tag="kt", name="kt")
                v_un[c] = lv.tile([128, 2, 256], BF, tag="vun", name="vun")
                for base, dst in ((0, qt[c]), (256, kt[c])):
                    for ot in range(2):
                        ps = psA.tile([128, C], F32, tag="psa")
                        for dt in range(8):
                            nc.tensor.matmul(
                                ps, lhsT=w[:, dt, base + ot * 128:base + (ot + 1) * 128],
                                rhs=hs[:, dt, :], start=(dt == 0), stop=(dt == 7))
                        th = wk.tile([128, C], BF, tag="th")
                        nc.scalar.activation(th, ps, AF.Tanh, scale=0.5)
                        nc.vector.scalar_tensor_tensor(
                            out=dst[:, ot, :], in0=th, scalar=1.0, in1=ps,
                            op0=OP.add, op1=OP.mult)
                for tt in range(2):
                    ps = psA.tile([128, 256], F32, tag="psa")
                    for dt in range(8):
                        nc.tensor.matmul(
                            ps, lhsT=hs[:, dt, tt * 128:(tt + 1) * 128],
                            rhs=w[:, dt, 512:768], start=(dt == 0), stop=(dt == 7))
                    th = wk.tile([128, 256], BF, tag="th")
                    nc.scalar.activation(th, ps, AF.Tanh, scale=0.5)
                    nc.vector.scalar_tensor_tensor(
                        out=v_un[c][:, tt, :], in0=th, scalar=1.0, in1=ps,
                        op0=OP.add, op1=OP.mult)

                # ---- transposes: st -> st_un [tau, m]; kt -> k_un [tau, dk] ----
                st_un[c] = lv.tile([128, 2, 256], BF, tag="stun", name="stun")
                k_un = wk.tile([128, 2, 256], BF, tag="kun")
                for lt in range(2):
                    pst = psT.tile([128, 256], BF, tag="pst")
                    for mt in range(2):
                        nc.tensor.transpose(
                            pst[:, mt * 128:(mt + 1) * 128],
                            st[c][:, mt, lt * 128:(lt + 1) * 128], ident)
                    nc.scalar.activation(st_un[c][:, lt, :], pst, AF.Copy)
                    psk = psT.tile([128, 256], BF, tag="pst")
                    for k2 in range(2):
                        nc.tensor.transpose(
                            psk[:, k2 * 128:(k2 + 1) * 128],
                            kt[c][:, k2, lt * 128:(lt + 1) * 128], ident)
                    nc.scalar.activation(k_un[:, lt, :], psk, AF.Copy)

                # ---- lend broadcast [p, m] (for Hk scale and Sh) ----
                lamcb = wk.tile([128, 2], BF, tag="lamcb")
                for mt in range(2):
                    nc.gpsimd.tensor_copy(lamcb[:, mt:mt + 1], lam[c][:, mt, C - 1:C])
                lrow = wk.tile([1, 256], BF, tag="lrow")
                for mt in range(2):
                    plr = psS.tile([1, 128], BF, tag="pss")
                    nc.tensor.transpose(plr, lamcb[:, mt:mt + 1], ident)
                    nc.vector.tensor_copy(lrow[0:1, mt * 128:(mt + 1) * 128], plr)
                pbc = psB.tile([128, 256], F32, tag="psb")
                nc.tensor.matmul(pbc, lhsT=ones_row, rhs=lrow, start=True, stop=True)
                lbc = wk.tile([128, 256], BF, tag="lbc")
                nc.vector.tensor_copy(lbc, pbc)

                # Sh = St * Lend (broadcast over tau partitions)
                sh_un = wk.tile([128, 2, 256], BF, tag="shun")
                for lt in range(2):
                    nc.vector.tensor_tensor(
                        sh_un[:, lt, :], st_un[c][:, lt, :], lbc, op=OP.mult)

                # ---- state updates; snapshots for the leaf chains ----
                last = (c == NCHUNK - 1)
                if not last:
                    hkb[c] = snp.tile([128, 2, 256], BF, tag="hkb", name="hkb")
                    hvb[c] = snp.tile([128, 2, 256], BF, tag="hvb", name="hvb")
                for dt2 in range(2):
                    ps = psA.tile([128, 256], F32, tag="psa")
                    for lt in range(2):
                        nc.tensor.matmul(
                            ps, lhsT=k_un[:, lt, dt2 * 128:(dt2 + 1) * 128],
                            rhs=sh_un[:, lt, :], start=(lt == 0), stop=(lt == 1))
                    tmp3 = wk.tile([128, 256], F32, tag="tmp3")
                    nc.gpsimd.tensor_tensor(tmp3, hk[:, dt2, :], lbc, op=OP.mult)
                    nc.vector.tensor_tensor(hk[:, dt2, :], tmp3, ps, op=OP.add)
                    if not last:
                        nc.scalar.activation(hkb[c][:, dt2, :], hk[:, dt2, :], AF.Copy)
                for mt in range(2):
                    ps = psA.tile([128, 256], F32, tag="psa")
                    for lt in range(2):
                        nc.tensor.matmul(
                            ps, lhsT=sh_un[:, lt, mt * 128:(mt + 1) * 128],
                            rhs=v_un[c][:, lt, :], start=(lt == 0), stop=(lt == 1))
                    nc.vector.scalar_tensor_tensor(
                        out=hv[:, mt, :], in0=hv[:, mt, :],
                        scalar=lam[c][:, mt, C - 1:C], in1=ps,
                        op0=OP.mult, op1=OP.add)
                    if not last:
                        nc.scalar.activation(hvb[c][:, mt, :], hv[:, mt, :], AF.Copy)

            def stage_Q(c):
                """Gram + ok + exponentials for chunk c (lag 1)."""
                ptm = wk.tile([128, 2, C], BF, tag="ptm")
                for lt in range(2):
                    ps = psA.tile([128, C], F32, tag="psa")
                    for k2 in range(2):
                        nc.tensor.matmul(
                            ps, lhsT=kt[c][:, k2, lt * 128:(lt + 1) * 128],
                            rhs=qt[c][:, k2, :], start=(k2 == 0), stop=(k2 == 1))
                    nc.vector.tensor_tensor(ptm[:, lt, :], ps, msk[:, lt, :],
                                            op=OP.mult)
                hkp = hkb[c - 1] if c > 0 else hkb0
                et[c] = lv.tile([128, 2, C], BF, tag="et", name="et")
                et2[c] = lv.tile([128, 2, C], BF, tag="et2", name="et2")
                for mt in range(2):
                    ps = psA.tile([128, C], F32, tag="psa")
                    for lt in range(2):
                        nc.tensor.matmul(
                            ps, lhsT=st_un[c][:, lt, mt * 128:(mt + 1) * 128],
                            rhs=ptm[:, lt, :], start=(lt == 0), stop=False)
                    for k2 in range(2):
                        nc.tensor.matmul(
                            ps, lhsT=hkp[:, k2, mt * 128:(mt + 1) * 128],
                            rhs=qt[c][:, k2, :], start=False, stop=(k2 == 1))
                    # silu-tanh factors: q,k carry 2x each -> ok_psum = 4*ok/lam
                    tmp = wk.tile([128, C], F32, tag="tmp")
                    nc.vector.tensor_tensor(tmp, lam[c][:, mt, :], ps, op=OP.mult)
                    w2 = wk.tile([128, C], F32, tag="w2")
                    nc.vector.scalar_tensor_tensor(
                        out=w2, in0=tmp, scalar=2.0, in1=S[c][:, mt, :],
                        op0=OP.mult, op1=OP.subtract)
                    nc.scalar.activation(et[c][:, mt, :], tmp, AF.Exp, scale=0.25)
                    # et2 = exp(ok)*lam  (exp(ok - S/8))
                    nc.scalar.activation(et2[c][:, mt, :], w2, AF.Exp, scale=0.125)

            def stage_R(c):
                """Softmax normalization for chunk c (lag 2)."""
                cs = psS.tile([1, C], F32, tag="pss")
                for mt in range(2):
                    nc.tensor.matmul(cs, lhsT=ones_col, rhs=et[c][:, mt, :],
                                     start=(mt == 0), stop=(mt == 1))
                rrow = wk.tile([1, C], BF, tag="rrow")
                with nc.allow_low_precision(reason="softmax denom bcast in bf16"):
                    nc.vector.reciprocal(rrow, cs)
                bcr = psB.tile([128, C], F32, tag="psb")
                nc.tensor.matmul(bcr, lhsT=ones_row, rhs=rrow, start=True, stop=True)
                qtt[c] = lv.tile([128, 2, C], BF, tag="qtt", name="qtt")
                for mt in range(2):
                    nc.vector.tensor_tensor(
                        qtt[c][:, mt, :], et2[c][:, mt, :], bcr, op=OP.mult)

            def stage_S(c):
                """Pass-2 output for chunk c (lag 3)."""
                p2m = wk.tile([128, 2, C], BF, tag="p2m")
                for lt in range(2):
                    ps = psA.tile([128, C], F32, tag="psa")
                    for mt in range(2):
                        nc.tensor.matmul(
                            ps, lhsT=st[c][:, mt, lt * 128:(lt + 1) * 128],
                            rhs=qtt[c][:, mt, :], start=(mt == 0), stop=(mt == 1))
                    nc.vector.tensor_tensor(p2m[:, lt, :], ps, msk[:, lt, :],
                                            op=OP.mult)
                hvp = hvb[c - 1] if c > 0 else hvb0
                zt = wk.tile([128, 2, C], BF, tag="zt")
                sq = wk.tile([128, 2, C], BF, tag="sq")
                for vt in range(2):
                    ps = psA.tile([128, C], F32, tag="psa")
                    for lt in range(2):
                        nc.tensor.matmul(
                            ps, lhsT=v_un[c][:, lt, 128 * vt:128 * (vt + 1)],
                            rhs=p2m[:, lt, :], start=(lt == 0), stop=False)
                    for mt in range(2):
                        nc.tensor.matmul(
                            ps, lhsT=hvp[:, mt, vt * 128:(vt + 1) * 128],
                            rhs=qtt[c][:, mt, :], start=False, stop=(mt == 1))
                    # ps = 2*o ; z = (tanh(o/2)+1)*2o = 4*silu(o)
                    th = wk.tile([128, C], BF, tag="th")
                    nc.scalar.activation(th, ps, AF.Tanh, scale=0.25)
                    nc.vector.scalar_tensor_tensor(
                        out=zt[:, vt, :], in0=th, scalar=1.0, in1=ps,
                        op0=OP.add, op1=OP.mult)
                    nc.gpsimd.tensor_tensor(sq[:, vt, :], zt[:, vt, :], zt[:, vt, :],
                                            op=OP.mult)
                nc.sync.dma_start(out=zv[:, :, c * C:(c + 1) * C], in_=zt)
                pq = psS.tile([1, C], F32, tag="pss")
                for vt in range(2):
                    nc.tensor.matmul(pq, lhsT=ones_col, rhs=sq[:, vt, :],
                                     start=(vt == 0), stop=(vt == 1))
                nc.vector.tensor_copy(pss_sb[0:1, c, :], pq)

            for it in range(NCHUNK + 3):
                if it < NCHUNK:
                    stage_P(it)
                if 1 <= it <= NCHUNK:
                    stage_Q(it - 1)
                if 2 <= it <= NCHUNK + 1:
                    stage_R(it - 2)
                if 3 <= it <= NCHUNK + 2:
                    stage_S(it - 3)
            nc.sync.dma_start(out=pss_d, in_=pss_sb.rearrange("p a t -> p (a t)"))
    nc.compile()
    return nc


def build_final():
    """Kernel 2: y = (z * rb) @ wot with z [1024, 512] bf16 feature-major,
    rb [1, 512] f32 precomputed rsqrt scalars. Out yT [1024, 512] bf16."""
    nc = bacc.Bacc("TRN2", target_bir_lowering=False, debug=False, num_devices=8)
    z_d = nc.dram_tensor("zin", [D, 512], BF, kind="ExternalInput").ap()
    wo_d = nc.dram_tensor("wot", [D, D], BF, kind="ExternalInput").ap()
    rb_d = nc.dram_tensor("rbin", [1, 512], F32, kind="ExternalInput").ap()
    y_d = nc.dram_tensor("y", [D, 512], BF, kind="ExternalOutput").ap()

    with tile.TileContext(nc) as tc:
        with (
            tc.tile_pool(name="sb", bufs=1) as sb,
            tc.tile_pool(name="yp", bufs=3) as yp,
            tc.tile_pool(name="ps", bufs=4, space="PSUM") as psp,
            tc.tile_pool(name="psb", bufs=1, space="PSUM") as psbp,
        ):
            z = sb.tile([128, 8, 512], BF, tag="z")
            wo = sb.tile([128, 8, 1024], BF, tag="wo")
            rb = sb.tile([1, 512], F32, tag="rb")
            ones_row = sb.tile([1, 128], F32, tag="onesrow")
            nc.sync.dma_start(out=rb, in_=rb_d)
            nc.sync.dma_start(out=z, in_=z_d.rearrange("(a p) t -> p a t", p=128))
            wov = wo_d.rearrange("(a p) o -> p a o", p=128)
            for ot in range(8):
                nc.sync.dma_start(out=wo[:, :, ot * 128:(ot + 1) * 128],
                                  in_=wov[:, :, ot * 128:(ot + 1) * 128])
            nc.vector.memset(ones_row, 1.0)

            bcr = psbp.tile([128, 512], F32, tag="bcr")
            nc.tensor.matmul(bcr, lhsT=ones_row, rhs=rb, start=True, stop=True)
            rbb = sb.tile([128, 512], F32, tag="rbb")
            nc.vector.tensor_copy(rbb, bcr)
            yv = y_d.rearrange("(a p) t -> p a t", p=128)
            for ot in range(8):
                ps = psp.tile([128, 512], F32, tag="ps")
                for ct in range(8):
                    nc.tensor.matmul(ps, lhsT=wo[:, ct, ot * 128:(ot + 1) * 128],
                                     rhs=z[:, ct, :], start=(ct == 0), stop=(ct == 7))
                ysb = yp.tile([128, 512], BF, tag="ysb")
                nc.vector.tensor_tensor(ysb, ps, rbb, op=OP.mult)
                nc.sync.dma_start(out=yv[:, ot, :], in_=ysb)
    nc.compile()
    return nc


def _get(name):
    if name not in _cache:
        _cache[name] = build_gsa() if name == "gsa" else build_final()
    return _cache[name]


def kernel(hidden_states, Wq, Wk, Wv, Wf, g_w, Wo, _trace=False):
    bf = ml_dtypes.bfloat16
    hidden_states = np.asarray(hidden_states, np.float32)
    Wq, Wk, Wv, Wf = (np.asarray(x, np.float32) for x in (Wq, Wk, Wv, Wf))
    g_w, Wo = np.asarray(g_w, np.float32), np.asarray(Wo, np.float32)

    mask = np.triu(np.ones((C, C), np.float32)).astype(bf)  # keep lambda <= tau
    ident = np.eye(128).astype(bf)
    in1 = []
    for core in range(8):
        b, h = core // 4, core % 4
        sl = slice(h * 256, (h + 1) * 256)
        wall = np.concatenate(
            [Wq[sl].T, Wk[sl].T, Wv[sl].T, Wf[sl].T], axis=1)   # [1024, 1024]
        in1.append({
            "hst": np.ascontiguousarray(hidden_states[b].T).astype(bf),
            "wall": np.ascontiguousarray(wall).astype(bf),
            "mask": mask,
            "ident": ident,
        })
    nc1 = _get("gsa")
    r1 = bass_utils.run_bass_kernel_spmd(nc1, in1, core_ids=list(range(8)),
                                         trace=_trace)
    zs = [r1.results[c]["z"] for c in range(8)]        # each [256, 2048] bf16
    pss = [np.asarray(r1.results[c]["pss"], np.float32) for c in range(8)]

    # Host glue: RMS rsqrt scalars from the device-side partial sums.
    # z = 4*silu(o)  =>  mean(silu^2) = pss_sum/(16*1024); the 1/4 de-scales z.
    wot = np.ascontiguousarray((Wo * g_w[None, :]).T).astype(bf)  # [c, o]
    rbs = []
    for b in range(B):
        ss = sum(pss[b * 4 + hh][0] for hh in range(4))           # [2048]
        rbs.append(0.25 / np.sqrt(ss / (16.0 * D) + EPS))         # [2048] f32
    in2 = []
    for core in range(8):
        b, q = core // 4, core % 4
        zb = np.concatenate([zs[b * 4 + hh] for hh in range(4)], axis=0)
        in2.append({
            "zin": np.ascontiguousarray(zb[:, q * 512:(q + 1) * 512]),
            "wot": wot,
            "rbin": np.ascontiguousarray(
                rbs[b][q * 512:(q + 1) * 512].reshape(1, 512).astype(np.float32)),
        })
    nc2 = _get("final")
    r2 = bass_utils.run_bass_kernel_spmd(nc2, in2, core_ids=list(range(8)),
                                         trace=_trace)
    out = np.empty((B, T, D), np.float32)
    for core in range(8):
        b, q = core // 4, core % 4
        out[b, q * 512:(q + 1) * 512, :] = np.asarray(
            r2.results[core]["y"], np.float32).T
    if _trace:
        kernel.last_traces = (r1, r2)
    return out


# revision 12
# speedup vs baseline: 1.5428x; 1.1271x over previous
"""Gated Slot Attention (GSA) Trainium2 kernel, v3.

Sharding: B*H = 8 lanes -> 8 cores (core = b*4 + h). Each core computes its
lane's projections + chunked two-pass GLA recurrence, emitting the raw lane
output z = 2*o transposed [DV, T]. A second kernel applies silu + RMSNorm +
output projection with rows of (b,t) split across cores.

Chunked recurrence (C=256, all within one lane):
  Lam[i,m] = prod_{j<=i} g[j,m]  (= exp(-cumsum(softplus(-xf))/8))
  rlam = 1/Lam ; st_t = s_t/Lam_t = rlam_t - rlam_{t-1}
  ok   = Lam*(q @ Hk + mask(k^T q)^T St); qv = softmax_m(ok); qtt = qv*Lam
  o    = qtt @ Hv + mask(St qtt)^T v
  Hk' = Lend*(Hk + k^T St) ; Hv' = Lend*(Hv + St^T v)   (Lend pulled out)

The state sequence (Hk_c, Hv_c) depends only on projections+gates, never on
the softmax path, so per-chunk state snapshots are computed ahead and each
chunk's softmax->output path is an independent leaf chain. Emission is
software-pipelined: all f-projections+gates first (one ln-table residency),
then per-chunk stages P (qkv/transposes/states), Q (ok/exp, lag 1),
R (softmax-norm, lag 2), S (pass-2 output, lag 3) under the exp table:
exactly 2 activation-table loads for the whole kernel.

silu is synthesized as 2*silu(x) = (tanh(x/2)+1)*x; the 2x factors on q,k
cancel via the exp scale, the 2x on v rides through to kernel 2 where the
tanh scale absorbs it and RMSNorm cancels the rest.
"""
import sys
sys.path.insert(0, '/opt/trn_rl_repo')

import numpy as np
import ml_dtypes

import concourse.bass as bass
import concourse.bacc as bacc
import concourse.tile as tile
import concourse.mybir as mybir
import concourse.bass_utils as bass_utils

BF = mybir.dt.bfloat16
F32 = mybir.dt.float32
AF = mybir.ActivationFunctionType
OP = mybir.AluOpType

B, T, D = 2, 2048, 1024
H, DK, DV, M = 4, 256, 256, 256
C = 256            # chunk length
NCHUNK = T // C
NBATCH = NCHUNK // 2   # 2-chunk projection batches
GATE_NORM = 8.0
EPS = 1e-5

_cache = {}


def build_gsa():
    """Kernel 1: per-lane projections + chunked GLA. Output z [256, 2048] bf16
    (= 2*o, feature-major)."""
    nc = bacc.Bacc("TRN2", target_bir_lowering=False, debug=False, num_devices=8)
    hsT_d = nc.dram_tensor("hst", [D, T], BF, kind="ExternalInput").ap()
    w_d = nc.dram_tensor("wall", [D, 4 * 256], BF, kind="ExternalInput").ap()
    mask_d = nc.dram_tensor("mask", [C, C], BF, kind="ExternalInput").ap()
    ident_d = nc.dram_tensor("ident", [128, 128], BF, kind="ExternalInput").ap()
    z_d = nc.dram_tensor("z", [DV, T], BF, kind="ExternalOutput").ap()

    with tile.TileContext(nc) as tc:
        with (
            tc.tile_pool(name="persist", bufs=1) as pp,
            tc.tile_pool(name="hsp", bufs=4) as hsp,
            tc.tile_pool(name="gb", bufs=2) as gb,      # gate short-lived (batch)
            tc.tile_pool(name="gk", bufs=NBATCH) as gk,  # gate kept (batch)
            tc.tile_pool(name="qk", bufs=2) as qkp,     # qt/kt batch tiles
            tc.tile_pool(name="lv", bufs=5) as lv,      # per-chunk leaf tensors
            tc.tile_pool(name="sn", bufs=4) as snp,     # state snapshots
            tc.tile_pool(name="wk", bufs=3) as wk,      # short-lived
            tc.tile_pool(name="p512", bufs=2, space="PSUM") as p512,
            tc.tile_pool(name="p256", bufs=3, space="PSUM") as p256,
            tc.tile_pool(name="pT", bufs=1, space="PSUM") as pT,
            tc.tile_pool(name="pS", bufs=1, space="PSUM") as pS,
            tc.tile_pool(name="pB", bufs=1, space="PSUM") as pB,
        ):
            w = pp.tile([128, 8, 1024], BF, tag="w")
            msk = pp.tile([128, 2, C], BF, tag="msk")
            ident = pp.tile([128, 128], BF, tag="ident")
            ones_col = pp.tile([128, 1], BF, tag="onescol")
            ones_row = pp.tile([1, 128], BF, tag="onesrow")
            hk = pp.tile([128, 2, 256], F32, tag="hk")
            hv = pp.tile([128, 2, 256], F32, tag="hv")
            hkb0 = pp.tile([128, 2, 256], BF, tag="hkb0")
            hvb0 = pp.tile([128, 2, 256], BF, tag="hvb0")

            wv = w_d.rearrange("(a p) o -> p a o", p=128)
            hsv = hsT_d.rearrange("(a p) t -> p a t", p=128)
            # f weights first: the gate phase runs before everything else.
            nc.sync.dma_start(out=w[:, :, 768:1024], in_=wv[:, :, 768:1024])
            hs_t = {}
            for bt in range(NBATCH):
                hs_t[bt] = hsp.tile([128, 8, 512], BF, tag="hs", name="hs")
                nc.sync.dma_start(out=hs_t[bt], in_=hsv[:, :, bt * 512:(bt + 1) * 512])
            nc.sync.dma_start(out=msk, in_=mask_d.rearrange("(a p) t -> p a t", p=128))
            nc.sync.dma_start(out=ident, in_=ident_d)
            nc.sync.dma_start(out=w[:, :, 0:768], in_=wv[:, :, 0:768])
            nc.vector.memset(ones_col, 1.0)
            nc.vector.memset(ones_row, 1.0)
            nc.gpsimd.memset(hk, 0.0)
            nc.gpsimd.memset(hv, 0.0)
            nc.gpsimd.memset(hkb0, 0.0)
            nc.gpsimd.memset(hvb0, 0.0)

            zv = z_d.rearrange("(a p) t -> p a t", p=128)

            Sb, lamb, stb, qtb, ktb = {}, {}, {}, {}, {}
            v_un, st_un, k_un, lbc, hkb, hvb, et, qtt = ({} for _ in range(8))

            # ---- phase F: f projections + gates for all batches (ln table) ----
            for bt in range(NBATCH):
                hs = hs_t[bt]
                e1 = gb.tile([128, 2, 512], F32, tag="e1", name="e1")
                rl = gb.tile([128, 2, 512], F32, tag="rl", name="rl")
                Sb[bt] = gk.tile([128, 2, 512], F32, tag="Sb", name="Sb")
                lamb[bt] = gk.tile([128, 2, 512], F32, tag="lamb", name="lamb")
                stb[bt] = gk.tile([128, 2, 512], BF, tag="stb", name="stb")
                for mt in range(2):
                    ps = p512.tile([128, 512], F32, tag="p512")
                    for dt in range(8):
                        nc.tensor.matmul(
                            ps, lhsT=w[:, dt, 768 + mt * 128:768 + (mt + 1) * 128],
                            rhs=hs[:, dt, :], start=(dt == 0), stop=(dt == 7))
                    nc.scalar.activation(e1[:, mt, :], ps, AF.Exp, scale=-1.0)
                    nc.vector.tensor_scalar_add(e1[:, mt, :], e1[:, mt, :], 1.0)
                    nc.scalar.activation(e1[:, mt, :], e1[:, mt, :], AF.Ln)
                    # e1 = nsp = softplus(-xf); per-chunk cumsum
                    nc.vector.tensor_tensor_scan(
                        Sb[bt][:, mt, 0:256], e1[:, mt, 0:256], e1[:, mt, 0:256],
                        0.0, OP.add, OP.bypass)
                    nc.vector.tensor_tensor_scan(
                        Sb[bt][:, mt, 256:512], e1[:, mt, 256:512],
                        e1[:, mt, 256:512], 0.0, OP.add, OP.bypass)
                    nc.scalar.activation(
                        rl[:, mt, :], Sb[bt][:, mt, :], AF.Exp, scale=1.0 / GATE_NORM)
                    nc.scalar.activation(
                        lamb[bt][:, mt, :], Sb[bt][:, mt, :], AF.Exp,
                        scale=-1.0 / GATE_NORM)
                    # st_t = rlam_t - rlam_{t-1}; chunk-boundary cols use rlam=1
                    nc.vector.tensor_tensor(
                        stb[bt][:, mt, 1:512], rl[:, mt, 1:512], rl[:, mt, 0:511],
                        op=OP.subtract)
                    for h2 in range(2):
                        nc.vector.tensor_scalar_sub(
                            stb[bt][:, mt, h2 * 256:h2 * 256 + 1],
                            rl[:, mt, h2 * 256:h2 * 256 + 1], 1.0)

            def chunk_views(c):
                bt, h2 = c // 2, c % 2
                off = h2 * 256
                stc = stb[bt][:, :, off:off + 256]
                lamc = lamb[bt][:, :, off:off + 256]
                qtc = qtb[bt][:, :, off:off + 256]
                ktc = ktb[bt][:, :, off:off + 256]
                return stc, lamc, qtc, ktc

            def stage_P(c):
                """qkv projections (even c), transposes, lend, state updates."""
                bt, h2 = c // 2, c % 2
                hs = hs_t[bt]
                if h2 == 0:
                    qtb[bt] = qkp.tile([128, 2, 512], BF, tag="qtb", name="qtb")
                    ktb[bt] = qkp.tile([128, 2, 512], BF, tag="ktb", name="ktb")
                    for base, dst in ((0, qtb[bt]), (256, ktb[bt])):
                        for ot in range(2):
                            ps = p512.tile([128, 512], F32, tag="p512")
                            for dt in range(8):
                                nc.tensor.matmul(
                                    ps,
                                    lhsT=w[:, dt, base + ot * 128:base + (ot + 1) * 128],
                                    rhs=hs[:, dt, :], start=(dt == 0), stop=(dt == 7))
                            th = wk.tile([128, 512], BF, tag="th")
                            nc.scalar.activation(th, ps, AF.Tanh, scale=0.5)
                            nc.vector.scalar_tensor_tensor(
                                out=dst[:, ot, :], in0=th, scalar=1.0, in1=ps,
                                op0=OP.add, op1=OP.mult)
                stc, lamc, qtc, ktc = chunk_views(c)

                # v untransposed [tau, dv] for this chunk
                v_un[c] = lv.tile([128, 2, 256], BF, tag="vun", name="vun")
                for tt in range(2):
                    ps = p256.tile([128, 256], F32, tag="p256")
                    for dt in range(8):
                        nc.tensor.matmul(
                            ps,
                            lhsT=hs[:, dt, h2 * 256 + tt * 128:h2 * 256 + (tt + 1) * 128],
                            rhs=w[:, dt, 512:768], start=(dt == 0), stop=(dt == 7))
                    th = wk.tile([128, 256], BF, tag="th2")
                    nc.scalar.activation(th, ps, AF.Tanh, scale=0.5)
                    nc.vector.scalar_tensor_tensor(
                        out=v_un[c][:, tt, :], in0=th, scalar=1.0, in1=ps,
                        op0=OP.add, op1=OP.mult)

                # transposes: st -> st_un [tau, m]; kt -> k_un [tau, dk]
                st_un[c] = lv.tile([128, 2, 256], BF, tag="stun", name="stun")
                k_un[c] = wk.tile([128, 2, 256], BF, tag="kun", name="kun")
                for lt in range(2):
                    pst = pT.tile([128, 256], BF, tag="pT")
                    for mt in range(2):
                        nc.tensor.transpose(
                            pst[:, mt * 128:(mt + 1) * 128],
                            stc[:, mt, lt * 128:(lt + 1) * 128], ident)
                    nc.scalar.activation(st_un[c][:, lt, :], pst, AF.Copy)
                    psk = pT.tile([128, 256], BF, tag="pT")
                    for k2 in range(2):
                        nc.tensor.transpose(
                            psk[:, k2 * 128:(k2 + 1) * 128],
                            ktc[:, k2, lt * 128:(lt + 1) * 128], ident)
                    nc.vector.tensor_copy(k_un[c][:, lt, :], psk)

                # lend broadcast [p, m]
                lamcb = wk.tile([128, 2], BF, tag="lamcb")
                for mt in range(2):
                    nc.gpsimd.tensor_copy(lamcb[:, mt:mt + 1], lamc[:, mt, 255:256])
                plr = pT.tile([128, 256], BF, tag="pT")
                for mt in range(2):
                    nc.tensor.transpose(
                        plr[0:1, mt * 128:(mt + 1) * 128], lamcb[:, mt:mt + 1], ident)
                lrow = wk.tile([1, 256], BF, tag="lrow")
                nc.vector.tensor_copy(lrow, plr[0:1, :])
                pbc = pB.tile([128, 256], F32, tag="pB")
                nc.tensor.matmul(pbc, lhsT=ones_row, rhs=lrow, start=True, stop=True)
                lbc[c] = wk.tile([128, 256], BF, tag="lbc", name="lbc")
                nc.vector.tensor_copy(lbc[c], pbc)

                # state updates: Hk' = Lend*(Hk + k^T St); Hv' = Lend*(Hv + St^T v)
                last = (c == NCHUNK - 1)
                if not last:
                    hkb[c] = snp.tile([128, 2, 256], BF, tag="hkb", name="hkb")
                    hvb[c] = snp.tile([128, 2, 256], BF, tag="hvb", name="hvb")
                for dt2 in range(2):
                    ps = p256.tile([128, 256], F32, tag="p256")
                    for lt in range(2):
                        nc.tensor.matmul(
                            ps, lhsT=k_un[c][:, lt, dt2 * 128:(dt2 + 1) * 128],
                            rhs=st_un[c][:, lt, :], start=(lt == 0), stop=(lt == 1))
                    nc.vector.tensor_tensor(hk[:, dt2, :], hk[:, dt2, :], ps,
                                            op=OP.add)
                    nc.gpsimd.tensor_tensor(hk[:, dt2, :], hk[:, dt2, :], lbc[c],
                                            op=OP.mult)
                    if not last:
                        nc.scalar.activation(hkb[c][:, dt2, :], hk[:, dt2, :], AF.Copy)
                for mt in range(2):
                    ps = p256.tile([128, 256], F32, tag="p256")
                    for lt in range(2):
                        nc.tensor.matmul(
                            ps, lhsT=st_un[c][:, lt, mt * 128:(mt + 1) * 128],
                            rhs=v_un[c][:, lt, :], start=(lt == 0), stop=(lt == 1))
                    nc.vector.tensor_tensor(hv[:, mt, :], hv[:, mt, :], ps, op=OP.add)
                    nc.vector.tensor_scalar_mul(hv[:, mt, :], hv[:, mt, :],
                                                 lamc[:, mt, 255:256])
                    if not last:
                        nc.scalar.activation(hvb[c][:, mt, :], hv[:, mt, :], AF.Copy)

            def stage_Q(c):
                """Gram + ok + exp for chunk c (lag 1)."""
                stc, lamc, qtc, ktc = chunk_views(c)
                ptm = wk.tile([128, 2, C], BF, tag="ptm")
                for lt in range(2):
                    ps = p256.tile([128, C], F32, tag="p256")
                    for k2 in range(2):
                        nc.tensor.matmul(
                            ps, lhsT=ktc[:, k2, lt * 128:(lt + 1) * 128],
                            rhs=qtc[:, k2, :], start=(k2 == 0), stop=(k2 == 1))
                    nc.vector.tensor_tensor(ptm[:, lt, :], ps, msk[:, lt, :],
                                            op=OP.mult)
                hkp = hkb[c - 1] if c > 0 else hkb0
                et[c] = lv.tile([128, 2, C], BF, tag="et", name="et")
                for mt in range(2):
                    ps = p256.tile([128, C], F32, tag="p256")
                    for lt in range(2):
                        nc.tensor.matmul(
                            ps, lhsT=st_un[c][:, lt, mt * 128:(mt + 1) * 128],
                            rhs=ptm[:, lt, :], start=(lt == 0), stop=False)
                    for k2 in range(2):
                        nc.tensor.matmul(
                            ps, lhsT=hkp[:, k2, mt * 128:(mt + 1) * 128],
                            rhs=qtc[:, k2, :], start=False, stop=(k2 == 1))
                    # q,k each carry 2x from the tanh-silu -> exp scale 0.25
                    tmp = wk.tile([128, C], F32, tag="tmp")
                    nc.vector.tensor_tensor(tmp, lamc[:, mt, :], ps, op=OP.mult)
                    nc.scalar.activation(et[c][:, mt, :], tmp, AF.Exp, scale=0.25)

            def stage_R(c):
                """Softmax normalization for chunk c (lag 2)."""
                stc, lamc, qtc, ktc = chunk_views(c)
                cs = pS.tile([1, C], F32, tag="pS")
                for mt in range(2):
                    nc.tensor.matmul(cs, lhsT=ones_col, rhs=et[c][:, mt, :],
                                     start=(mt == 0), stop=(mt == 1))
                rrow = wk.tile([1, C], BF, tag="rrow")
                with nc.allow_low_precision(reason="softmax denom bcast in bf16"):
                    nc.vector.reciprocal(rrow, cs)
                bcr = pB.tile([128, C], F32, tag="pB")
                nc.tensor.matmul(bcr, lhsT=ones_row, rhs=rrow, start=True, stop=True)
                qtt[c] = lv.tile([128, 2, C], BF, tag="qtt", name="qtt")
                tmp2 = wk.tile([128, 2, C], BF, tag="tmp2")
                for mt in range(2):
                    nc.gpsimd.tensor_tensor(tmp2[:, mt, :], lamc[:, mt, :],
                                            et[c][:, mt, :], op=OP.mult)
                    nc.vector.tensor_tensor(qtt[c][:, mt, :], tmp2[:, mt, :], bcr,
                                            op=OP.mult)

            def stage_S(c):
                """Pass-2 output for chunk c (lag 3)."""
                stc, lamc, qtc, ktc = chunk_views(c)
                p2m = wk.tile([128, 2, C], BF, tag="p2m")
                for lt in range(2):
                    ps = p256.tile([128, C], F32, tag="p256")
                    for mt in range(2):
                        nc.tensor.matmul(
                            ps, lhsT=stc[:, mt, lt * 128:(lt + 1) * 128],
                            rhs=qtt[c][:, mt, :], start=(mt == 0), stop=(mt == 1))
                    nc.vector.tensor_tensor(p2m[:, lt, :], ps, msk[:, lt, :],
                                            op=OP.mult)
                hvp = hvb[c - 1] if c > 0 else hvb0
                zt = wk.tile([128, 2, C], BF, tag="zt")
                for vt in range(2):
                    ps = p256.tile([128, C], F32, tag="p256")
                    for lt in range(2):
                        nc.tensor.matmul(
                            ps, lhsT=v_un[c][:, lt, 128 * vt:128 * (vt + 1)],
                            rhs=p2m[:, lt, :], start=(lt == 0), stop=False)
                    for mt in range(2):
                        nc.tensor.matmul(
                            ps, lhsT=hvp[:, mt, vt * 128:(vt + 1) * 128],
                            rhs=qtt[c][:, mt, :], start=False, stop=(mt == 1))
                    nc.scalar.activation(zt[:, vt, :], ps, AF.Copy)
                nc.sync.dma_start(out=zv[:, :, c * C:(c + 1) * C], in_=zt)

            for it in range(NCHUNK + 3):
                if it < NCHUNK:
                    stage_P(it)
                if 1 <= it <= NCHUNK:
                    stage_Q(it - 1)
                if 2 <= it <= NCHUNK + 1:
                    stage_R(it - 2)
                if 3 <= it <= NCHUNK + 2:
                    stage_S(it - 3)
    nc.compile()
    return nc


def build_final():
    """Kernel 2: u = 4*silu(z/2); y = (u * rsqrt-bcast) @ wot.
    z [1024, 512] bf16 feature-major; wot host-folded (g_w, 1/4).
    Out yT [1024, 512] bf16."""
    nc = bacc.Bacc("TRN2", target_bir_lowering=False, debug=False, num_devices=8)
    z_d = nc.dram_tensor("zin", [D, 512], BF, kind="ExternalInput").ap()
    wo_d = nc.dram_tensor("wot", [D, D], BF, kind="ExternalInput").ap()
    y_d = nc.dram_tensor("y", [D, 512], BF, kind="ExternalOutput").ap()

    with tile.TileContext(nc) as tc:
        with (
            tc.tile_pool(name="sb", bufs=1) as sb,
            tc.tile_pool(name="yp", bufs=3) as yp,
            tc.tile_pool(name="ps", bufs=4, space="PSUM") as psp,
            tc.tile_pool(name="pss", bufs=1, space="PSUM") as pssp,
            tc.tile_pool(name="psb", bufs=1, space="PSUM") as psbp,
        ):
            z = sb.tile([128, 8, 512], BF, tag="z")
            wo = sb.tile([128, 8, 1024], BF, tag="wo")
            u = sb.tile([128, 8, 512], BF, tag="u")
            squ = sb.tile([128, 8, 512], BF, tag="squ")
            ones_col = sb.tile([128, 1], BF, tag="onescol")
            ones_row = sb.tile([1, 128], BF, tag="onesrow")
            nc.vector.memset(ones_col, 1.0)
            nc.vector.memset(ones_row, 1.0)
            zvw = z_d.rearrange("(a p) t -> p a t", p=128)
            for ct in range(8):
                nc.sync.dma_start(out=z[:, ct, :], in_=zvw[:, ct, :])
            wov = wo_d.rearrange("(a p) o -> p a o", p=128)
            for ct in range(8):
                nc.sync.dma_start(out=wo[:, ct, :], in_=wov[:, ct, :])

            # silu: z = 2*o ; u = (tanh(o/2)+1)*z = 4*silu(o)
            sqs = pssp.tile([1, 512], F32, tag="pss")
            for ct in range(8):
                th = yp.tile([128, 512], BF, tag="th")
                nc.scalar.activation(th, z[:, ct, :], AF.Tanh, scale=0.25)
                nc.vector.scalar_tensor_tensor(
                    out=u[:, ct, :], in0=th, scalar=1.0, in1=z[:, ct, :],
                    op0=OP.add, op1=OP.mult)
                nc.gpsimd.tensor_tensor(squ[:, ct, :], u[:, ct, :], u[:, ct, :],
                                        op=OP.mult)
                nc.tensor.matmul(sqs, lhsT=ones_col, rhs=squ[:, ct, :],
                                 start=(ct == 0), stop=(ct == 7))
            # u = 4*silu -> mean(silu^2) = sqs/(16*1024); the 1/4 is in wot
            sq = sb.tile([1, 512], F32, tag="sq")
            eps_t = sb.tile([1, 1], F32, tag="epst")
            nc.vector.memset(eps_t, EPS)
            nc.scalar.activation(sq, sqs, AF.Sqrt, scale=1.0 / (16.0 * D), bias=eps_t)
            rr = sb.tile([1, 512], BF, tag="rr")
            with nc.allow_low_precision(reason="rms bcast in bf16"):
                nc.vector.reciprocal(rr, sq)
            bcr = psbp.tile([128, 512], F32, tag="psb")
            nc.tensor.matmul(bcr, lhsT=ones_row, rhs=rr, start=True, stop=True)
            rbb = sb.tile([128, 512], F32, tag="rbb")
            nc.vector.tensor_copy(rbb, bcr)

            yv = y_d.rearrange("(a p) t -> p a t", p=128)
            for g in range(2):
                yps = [psp.tile([128, 512], F32, tag="ps", name="yps")
                       for _ in range(4)]
                for ct in range(8):
                    for oi in range(4):
                        ot = g * 4 + oi
                        nc.tensor.matmul(
                            yps[oi], lhsT=wo[:, ct, ot * 128:(ot + 1) * 128],
                            rhs=u[:, ct, :], start=(ct == 0), stop=(ct == 7))
                for oi in range(4):
                    ot = g * 4 + oi
                    ysb = yp.tile([128, 512], BF, tag="ysb")
                    nc.vector.tensor_tensor(ysb, yps[oi], rbb, op=OP.mult)
                    nc.sync.dma_start(out=yv[:, ot, :], in_=ysb)
    nc.compile()
    return nc


def _get(name):
    if name not in _cache:
        _cache[name] = build_gsa() if name == "gsa" else build_final()
    return _cache[name]


def kernel(hidden_states, Wq, Wk, Wv, Wf, g_w, Wo, _trace=False):
    bf = ml_dtypes.bfloat16
    hidden_states = np.asarray(hidden_states, np.float32)
    Wq, Wk, Wv, Wf = (np.asarray(x, np.float32) for x in (Wq, Wk, Wv, Wf))
    g_w, Wo = np.asarray(g_w, np.float32), np.asarray(Wo, np.float32)

    mask = np.triu(np.ones((C, C), np.float32)).astype(bf)  # keep lambda <= tau
    ident = np.eye(128).astype(bf)
    in1 = []
    for core in range(8):
        b, h = core // 4, core % 4
        sl = slice(h * 256, (h + 1) * 256)
        wall = np.concatenate(
            [Wq[sl].T, Wk[sl].T, Wv[sl].T, Wf[sl].T], axis=1)   # [1024, 1024]
        in1.append({
            "hst": np.ascontiguousarray(hidden_states[b].T).astype(bf),
            "wall": np.ascontiguousarray(wall).astype(bf),
            "mask": mask,
            "ident": ident,
        })
    nc1 = _get("gsa")
    r1 = bass_utils.run_bass_kernel_spmd(nc1, in1, core_ids=list(range(8)),
                                         trace=_trace)
    zs = [r1.results[c]["z"] for c in range(8)]        # each [256, 2048] bf16

    # wot folds g_w and the 1/4 that de-scales u = 4*silu(o)
    wot = np.ascontiguousarray((Wo * (0.25 * g_w)[None, :]).T).astype(bf)
    in2 = []
    for core in range(8):
        b, q = core // 4, core % 4
        zb = np.concatenate([zs[b * 4 + hh] for hh in range(4)], axis=0)
        in2.append({
            "zin": np.ascontiguousarray(zb[:, q * 512:(q + 1) * 512]),
            "wot": wot,
        })
    nc2 = _get("final")
    r2 = bass_utils.run_bass_kernel_spmd(nc2, in2, core_ids=list(range(8)),
                                         trace=_trace)
    out = np.empty((B, T, D), np.float32)
    for core in range(8):
        b, q = core // 4, core % 4
        out[b, q * 512:(q + 1) * 512, :] = np.asarray(
            r2.results[core]["y"], np.float32).T
    if _trace:
        kernel.last_traces = (r1, r2)
    return out
